# revision 16
# baseline (speedup 1.0000x reference)
"""AuxSeLoss v7: bf16, engine-balanced softplus via sigmoid+relu split.

Math: t in {0,1} exactly -> per-element BCE = softplus(z), z = (1-2t)x.
softplus(z) = relu(z) + g(|z|), g(u) = log1p(exp(-u)) ~= A*sigmoid(-B*u+G)+D
(minimax fit, max |err| 4.9e-4).  With z'' = (t-0.5)x (so z = -2z''):
relu(z) = rn := max(-2z'', 0), |z''| = z'' + rn.

Engine balance (HW-measured modes: TT=2x, plain TS=4x, any DVE accum=1x,
ACT=1/cyc with free exact accumulator, PE chain mm ~400ns/512 cols):
  DVE per chunk: s_t = t-0.5 [ts F/4]; per tensor: z''=s_t*x in place
    [tt F/2], rn on the back (1-rho) cols [ts 2-op], a=z''+rn in place
    [tt F/2], sigma in place [ACT].
  ACT: per tensor one full-width Sigmoid pass (scale=-2B, accum) plus a
    Relu pass (scale=-2, accum) on the front rho~1/3 cols - writing the
    same rn tile slice DVE skips; its accumulator supplies that slice's
    sum(relu(z)) exactly.
  PE: ones-chains (512 blocks) for per-sample sum(t) and the back-cols
    sum(rn); per-chunk [P,16] Vc stat fold.
  DMA: t rides the ACT HWDGE queue (nc.scalar), x0/x1 the sync queue, so
    the t tile for chunk c+1 never waits behind x1 of chunk c.
Host: bf16 cast + O(1) combine.  rel err ~1e-4 (gate 2e-2).
"""

import numpy as np
import ml_dtypes

N_CLASSES = 21
B, C, H, W = 16, N_CLASSES, 256, 256
N_CORES = 8
B_LOCAL = B // N_CORES
ELEMS_PER_SAMPLE = C * H * W  # 1376256
P = 128
FREE_PER_SAMPLE = ELEMS_PER_SAMPLE // P  # 10752
ROWS = B_LOCAL * P
AUX_WEIGHT = 0.4
SE_WEIGHT = 0.2
N_TOTAL = B * C * H * W
N_SE = B * C
N_CORE_T = B_LOCAL * ELEMS_PER_SAMPLE  # elems per tensor per core

# minimax fit of log1p(exp(-u)) ~= ALPHA*sigmoid(-BETA*u+GAMMA)+DELTA, u>=0
ALPHA = 2.49059269
BETA = 0.985901754
GAMMA = -0.954224925
DELTA = 1.79845165e-4

# (cols, act_relu_cols) per chunk; act cols ride ACT, rest DVE+PE.
CHUNK_SCHEDULE = [
    [(512, 0), (2560, 512), (3072, 1024), (4608, 1536)],
    [(5632, 1536), (4608, 1024), (512, 0)],
]
assert all(sum(c for c, _ in cs) == FREE_PER_SAMPLE for cs in CHUNK_SCHEDULE)
assert all(c % 512 == 0 and k % 512 == 0 and k < c
           for cs in CHUNK_SCHEDULE for c, k in cs)
N_CHUNKS0 = len(CHUNK_SCHEDULE[0])
FALLOC = 5776  # 5632 max cols + 288B pad so bank phases differ
TBLK = 512  # PE chain block width (one PSUM bank of f32)

_CACHE: dict = {}


def _build():
    from contextlib import ExitStack

    import concourse.bacc as bacc
    import concourse.mybir as mybir
    from concourse.tile import TileContext

    f32 = mybir.dt.float32
    bf16 = mybir.dt.bfloat16
    AFT = mybir.ActivationFunctionType
    ALU = mybir.AluOpType

    nc = bacc.Bacc("TRN2", target_bir_lowering=False)
    x0 = nc.dram_tensor("out0", [ROWS, FREE_PER_SAMPLE], bf16, kind="ExternalInput")
    x1 = nc.dram_tensor("out1", [ROWS, FREE_PER_SAMPLE], bf16, kind="ExternalInput")
    tg = nc.dram_tensor("targets", [ROWS, FREE_PER_SAMPLE], bf16, kind="ExternalInput")
    o2 = nc.dram_tensor("out2", [1, B_LOCAL * C], f32, kind="ExternalInput")
    res = nc.dram_tensor("stats", [1, 16], f32, kind="ExternalOutput")

    with ExitStack() as ctx, TileContext(nc) as tc:
        with (
            tc.tile_pool(name="x0p", bufs=3) as x0p,
            tc.tile_pool(name="x1p", bufs=3) as x1p,
            tc.tile_pool(name="tp", bufs=3) as tp,
            tc.tile_pool(name="sp", bufs=2) as sp,
            tc.tile_pool(name="rnp", bufs=4) as rnp,
            tc.tile_pool(name="vcp", bufs=2) as vcp,
            tc.tile_pool(name="accp", bufs=1) as accp,
            tc.tile_pool(name="psp", bufs=1, space="PSUM") as psp,
        ):
            ones_f = accp.tile([P, 1], f32)
            nc.vector.memset(ones_f[:], 1.0)
            ones_b = accp.tile([P, 1], bf16)
            nc.vector.memset(ones_b[:], 1.0)
            gam_t = accp.tile([P, 1], f32)
            nc.vector.memset(gam_t[:], GAMMA)
            Us = accp.tile([1, 16], f32)
            o2_t = accp.tile([1, B_LOCAL * C], f32)
            a_o2 = accp.tile([1, B_LOCAL * C], f32)
            r_o2 = accp.tile([1, B_LOCAL * C], f32)
            U = psp.tile([1, 16], f32)
            warm = psp.tile([1, 16], f32)
            pt = [psp.tile([1, TBLK], f32, name=f"pt{s}") for s in range(B_LOCAL)]
            pr = [psp.tile([1, TBLK], f32, name=f"pr{i}") for i in range(2)]

            chunks = []
            for s in range(B_LOCAL):
                c0 = 0
                for cols, ka in CHUNK_SCHEDULE[s]:
                    chunks.append((s, c0, cols, ka))
                    c0 += cols
            n_chunks = len(chunks)
            # total PE rn-chain cols per tensor (back cols of every chunk)
            rtot = sum(c - k for cs in CHUNK_SCHEDULE for c, k in cs)
            rcols = [0, 0]

            # HAM warm-up: ~4.5us of back-to-back tiny matmuls while the
            # DMA pipe fills; flips the PE clock gate to 8/8 (2.4 GHz)
            # before the real ones-chains start.
            for w in range(140):
                nc.tensor.matmul(warm[0:1, 0:1], ones_f[:], gam_t[:],
                                 start=True, stop=True)

            first = True
            for ci, (s, c0, cols, ka) in enumerate(chunks):
                r0, r1 = s * P, (s + 1) * P
                c1 = c0 + cols
                t_t = tp.tile([P, FALLOC], bf16, name=f"t_{ci}", tag="t")
                x0_t = x0p.tile([P, FALLOC], bf16, name=f"x0_{ci}", tag="x0")
                x1_t = x1p.tile([P, FALLOC], bf16, name=f"x1_{ci}", tag="x1")
                nc.sync.dma_start(t_t[:, 0:cols], tg[r0:r1, c0:c1])
                nc.sync.dma_start(x0_t[:, 0:cols], x0[r0:r1, c0:c1])
                nc.sync.dma_start(x1_t[:, 0:cols], x1[r0:r1, c0:c1])

                Vc = vcp.tile([P, 16], f32, name=f"vc_{ci}", tag="vc")
                nc.gpsimd.memset(Vc[:], 0.0)

                if first:
                    # out2 warmup: tiny abs+sigmoid forces the sigmoid
                    # table load before the first bulk ACT op.
                    nc.sync.dma_start(o2_t[:], o2[0:1, :])
                    nc.vector.scalar_tensor_tensor(
                        out=a_o2[:], in0=o2_t[:], scalar=-1.0, in1=o2_t[:],
                        op0=ALU.mult, op1=ALU.max,
                    )
                    nc.vector.tensor_scalar(
                        out=r_o2[:], in0=o2_t[:], scalar1=0.0, scalar2=None,
                        op0=ALU.max, op1=ALU.add,
                        accum_out=Vc[0:1, 4:5],
                    )
                    nc.scalar.activation(
                        a_o2[:], a_o2[:], AFT.Sigmoid,
                        bias=gam_t[0:1], scale=-BETA,
                        accum_out=Vc[0:1, 5:6],
                    )
                    first = False

                s_t = sp.tile([P, FALLOC], bf16, name=f"s_{ci}", tag="s")
                nc.vector.tensor_scalar(
                    out=s_t[:, 0:cols], in0=t_t[:, 0:cols], scalar1=0.5,
                    scalar2=None, op0=ALU.subtract,
                )
                xts = (x0_t, x1_t)
                rn_ts = [rnp.tile([P, FALLOC], bf16, name=f"rn{i}_{ci}", tag="rn")
                         for i in range(2)]
                # phase 1 (DVE): z'' = s_t * x, in place over x
                for i, xt in enumerate(xts):
                    nc.vector.tensor_tensor(
                        xt[:, 0:cols], s_t[:, 0:cols], xt[:, 0:cols], ALU.mult
                    )
                # phase 2 (ACT): rn = relu(-2 z'') on front ka cols with an
                # exact fused sum; (DVE): same on back cols via 2-op ts.
                for i, xt in enumerate(xts):
                    if ka:
                        nc.scalar.activation(
                            rn_ts[i][:, 0:ka], xt[:, 0:ka], AFT.Relu,
                            scale=-2.0, accum_out=Vc[:, 2 + i : 3 + i],
                        )
                for i, xt in enumerate(xts):
                    nc.vector.tensor_scalar(
                        out=rn_ts[i][:, ka:cols], in0=xt[:, ka:cols],
                        scalar1=-2.0, scalar2=0.0, op0=ALU.mult, op1=ALU.max,
                    )
                # phase 3: a = z'' + rn = |z''| in place; then sigma in place
                for i, xt in enumerate(xts):
                    nc.vector.tensor_tensor(
                        xt[:, 0:cols], xt[:, 0:cols], rn_ts[i][:, 0:cols],
                        ALU.add
                    )
                    nc.scalar.activation(
                        xt[:, 0:cols], xt[:, 0:cols], AFT.Sigmoid,
                        bias=gam_t[:], scale=-2.0 * BETA,
                        accum_out=Vc[:, i : i + 1],
                    )
                # phase 4 (PE): chains over the back rn cols
                for i in range(2):
                    for j in range(ka, cols, TBLK):
                        nc.tensor.matmul(
                            pr[i][:], ones_b[:], rn_ts[i][:, j : j + TBLK],
                            start=(rcols[i] == 0),
                            stop=(rcols[i] + TBLK == rtot),
                        )
                        rcols[i] += TBLK

                # PE chain: per-sample sum(t)
                for j in range(cols // TBLK):
                    off = c0 + j * TBLK
                    nc.tensor.matmul(
                        pt[s][:], ones_b[:], t_t[:, j * TBLK : (j + 1) * TBLK],
                        start=(off == 0), stop=(off + TBLK == FREE_PER_SAMPLE),
                    )

                # fold this chunk's stats into PSUM (fp32r ones-matmul)
                nc.tensor.matmul(
                    U[:], ones_f[:], Vc[:],
                    start=(ci == 0), stop=(ci == n_chunks - 1),
                )

                if ci == N_CHUNKS0 - 1:
                    # sample 0's t-chain just stopped; drain it early
                    nc.vector.tensor_reduce(
                        out=Us[0:1, 8:9], in_=pt[0][:],
                        axis=mybir.AxisListType.X, op=ALU.add,
                    )

            nc.vector.tensor_reduce(
                out=Us[0:1, 9:10], in_=pt[1][:],
                axis=mybir.AxisListType.X, op=ALU.add,
            )
            for i in range(2):
                nc.vector.tensor_reduce(
                    out=Us[0:1, 10 + i : 11 + i], in_=pr[i][:],
                    axis=mybir.AxisListType.X, op=ALU.add,
                )
            nc.vector.tensor_copy(Us[0:1, 0:8], U[0:1, 0:8])
            nc.sync.dma_start(res[0:1, :], Us[:])

    nc.finalize()
    return nc


def _get_nc():
    if "nc" not in _CACHE:
        _CACHE["nc"] = _build()
    return _CACHE["nc"]


def _run(in_maps, trace=False):
    from concourse.bass_utils import run_bass_kernel_spmd

    return run_bass_kernel_spmd(
        _get_nc(), in_maps, core_ids=list(range(N_CORES)), trace=trace
    )


def make_in_maps(out0, out1, out2, targets):
    bf = ml_dtypes.bfloat16
    out0 = np.asarray(out0, dtype=np.float32).astype(bf)
    out1 = np.asarray(out1, dtype=np.float32).astype(bf)
    targets = np.asarray(targets, dtype=np.float32).astype(bf)
    out2 = np.asarray(out2, dtype=np.float32)
    in_maps = []
    for c in range(N_CORES):
        sl = slice(c * B_LOCAL, (c + 1) * B_LOCAL)
        in_maps.append(
            {
                "out0": np.ascontiguousarray(out0[sl]).reshape(ROWS, FREE_PER_SAMPLE),
                "out1": np.ascontiguousarray(out1[sl]).reshape(ROWS, FREE_PER_SAMPLE),
                "targets": np.ascontiguousarray(targets[sl]).reshape(
                    ROWS, FREE_PER_SAMPLE
                ),
                "out2": np.ascontiguousarray(out2[sl]).reshape(1, B_LOCAL * C),
            }
        )
    return in_maps


def combine_partials(stats, out2):
    """Host-side O(1) combine. stats: [N_CORES, 16] per-core sums.
    cols: 0,1=sum sig per tensor; 2,3=ACT-side sum relu(z) per tensor;
    4=sum relu(o2); 5=sum sig(o2); 8,9=sum(t) per sample;
    10,11=PE-side sum relu(z) per tensor."""
    total_main = 0.0
    total_se = 0.0
    for c in range(len(stats)):
        v = [float(x) for x in stats[c]]
        s0 = v[2] + v[10] + ALPHA * v[0] + DELTA * N_CORE_T
        s1 = v[3] + v[11] + ALPHA * v[1] + DELTA * N_CORE_T
        total_main += s0 + AUX_WEIGHT * s1
        total_se += v[4] + ALPHA * v[5] + DELTA * (B_LOCAL * C)
        for s in range(B_LOCAL):
            t_sum = v[8 + s]
            b_global = c * B_LOCAL + s
            if t_sum < ELEMS_PER_SAMPLE - 0.5:  # class-bin 0 present
                total_se -= float(out2[b_global, 0])
            if t_sum > 0.5:  # class-bin 1 present
                total_se -= float(out2[b_global, 1])
    return total_main / N_TOTAL + SE_WEIGHT * total_se / N_SE


def kernel(out0, out1, out2, targets):
    out2 = np.asarray(out2, dtype=np.float32)
    br = _run(make_in_maps(out0, out1, out2, targets))
    stats = [r["stats"][0] for r in br.results]
    return np.asarray(combine_partials(stats, out2), dtype=np.float32)


# revision 17
# speedup vs baseline: 1.0097x; 1.0097x over previous
"""AuxSeLoss v7: bf16, engine-balanced softplus via sigmoid+relu split.

Math: t in {0,1} exactly -> per-element BCE = softplus(z), z = (1-2t)x.
softplus(z) = relu(z) + g(|z|), g(u) = log1p(exp(-u)) ~= A*sigmoid(-B*u+G)+D
(minimax fit, max |err| 4.9e-4).  With z'' = (t-0.5)x (so z = -2z''):
relu(z) = rn := max(-2z'', 0), |z''| = z'' + rn.

Engine balance (HW-measured modes: TT=2x, plain TS=4x, any DVE accum=1x,
ACT=1/cyc with free exact accumulator, PE chain mm ~400ns/512 cols):
  DVE per chunk: s_t = t-0.5 [ts F/4]; per tensor: z''=s_t*x in place
    [tt F/2], rn on the back (1-rho) cols [ts 2-op], a=z''+rn in place
    [tt F/2], sigma in place [ACT].
  ACT: per tensor one full-width Sigmoid pass (scale=-2B, accum) plus a
    Relu pass (scale=-2, accum) on the front rho~1/3 cols - writing the
    same rn tile slice DVE skips; its accumulator supplies that slice's
    sum(relu(z)) exactly.
  PE: ones-chains (512 blocks) for per-sample sum(t) and the back-cols
    sum(rn); per-chunk [P,16] Vc stat fold.
  DMA: t rides the ACT HWDGE queue (nc.scalar), x0/x1 the sync queue, so
    the t tile for chunk c+1 never waits behind x1 of chunk c.
Host: bf16 cast + O(1) combine.  rel err ~1e-4 (gate 2e-2).
"""

import numpy as np
import ml_dtypes

N_CLASSES = 21
B, C, H, W = 16, N_CLASSES, 256, 256
N_CORES = 8
B_LOCAL = B // N_CORES
ELEMS_PER_SAMPLE = C * H * W  # 1376256
P = 128
FREE_PER_SAMPLE = ELEMS_PER_SAMPLE // P  # 10752
ROWS = B_LOCAL * P
AUX_WEIGHT = 0.4
SE_WEIGHT = 0.2
N_TOTAL = B * C * H * W
N_SE = B * C
N_CORE_T = B_LOCAL * ELEMS_PER_SAMPLE  # elems per tensor per core

# minimax fit of log1p(exp(-u)) ~= ALPHA*sigmoid(-BETA*u+GAMMA)+DELTA, u>=0
ALPHA = 2.49059269
BETA = 0.985901754
GAMMA = -0.954224925
DELTA = 1.79845165e-4

# (cols, act_relu_cols) per chunk; act cols ride ACT, rest DVE+PE.
CHUNK_SCHEDULE = [
    [(1024, 0), (2048, 512), (3072, 1024), (4608, 1536)],
    [(5632, 1536), (4608, 1024), (512, 0)],
]
assert all(sum(c for c, _ in cs) == FREE_PER_SAMPLE for cs in CHUNK_SCHEDULE)
assert all(c % 512 == 0 and k % 512 == 0 and k < c
           for cs in CHUNK_SCHEDULE for c, k in cs)
N_CHUNKS0 = len(CHUNK_SCHEDULE[0])
FALLOC = 5776  # 5632 max cols + 288B pad so bank phases differ
TBLK = 512  # PE chain block width (one PSUM bank of f32)

_CACHE: dict = {}


def _build():
    from contextlib import ExitStack

    import concourse.bacc as bacc
    import concourse.mybir as mybir
    from concourse.tile import TileContext

    f32 = mybir.dt.float32
    bf16 = mybir.dt.bfloat16
    AFT = mybir.ActivationFunctionType
    ALU = mybir.AluOpType

    nc = bacc.Bacc("TRN2", target_bir_lowering=False)
    x0 = nc.dram_tensor("out0", [ROWS, FREE_PER_SAMPLE], bf16, kind="ExternalInput")
    x1 = nc.dram_tensor("out1", [ROWS, FREE_PER_SAMPLE], bf16, kind="ExternalInput")
    tg = nc.dram_tensor("targets", [ROWS, FREE_PER_SAMPLE], bf16, kind="ExternalInput")
    o2 = nc.dram_tensor("out2", [1, B_LOCAL * C], f32, kind="ExternalInput")
    res = nc.dram_tensor("stats", [1, 16], f32, kind="ExternalOutput")

    with ExitStack() as ctx, TileContext(nc) as tc:
        with (
            tc.tile_pool(name="x0p", bufs=3) as x0p,
            tc.tile_pool(name="x1p", bufs=3) as x1p,
            tc.tile_pool(name="tp", bufs=3) as tp,
            tc.tile_pool(name="sp", bufs=2) as sp,
            tc.tile_pool(name="rnp", bufs=4) as rnp,
            tc.tile_pool(name="vcp", bufs=2) as vcp,
            tc.tile_pool(name="accp", bufs=1) as accp,
            tc.tile_pool(name="psp", bufs=1, space="PSUM") as psp,
        ):
            ones_f = accp.tile([P, 1], f32)
            nc.vector.memset(ones_f[:], 1.0)
            ones_b = accp.tile([P, 1], bf16)
            nc.vector.memset(ones_b[:], 1.0)
            gam_t = accp.tile([P, 1], f32)
            nc.vector.memset(gam_t[:], GAMMA)
            Us = accp.tile([1, 16], f32)
            o2_t = accp.tile([1, B_LOCAL * C], f32)
            a_o2 = accp.tile([1, B_LOCAL * C], f32)
            r_o2 = accp.tile([1, B_LOCAL * C], f32)
            U = psp.tile([1, 16], f32)
            warm = psp.tile([1, 16], f32)
            pt = [psp.tile([1, TBLK], f32, name=f"pt{s}") for s in range(B_LOCAL)]
            pr = [psp.tile([1, TBLK], f32, name=f"pr{i}") for i in range(2)]

            chunks = []
            for s in range(B_LOCAL):
                c0 = 0
                for cols, ka in CHUNK_SCHEDULE[s]:
                    chunks.append((s, c0, cols, ka))
                    c0 += cols
            n_chunks = len(chunks)
            # total PE rn-chain cols per tensor (back cols of every chunk)
            rtot = sum(c - k for cs in CHUNK_SCHEDULE for c, k in cs)
            rcols = [0, 0]

            # HAM warm-up: ~4.5us of back-to-back tiny matmuls while the
            # DMA pipe fills; flips the PE clock gate to 8/8 (2.4 GHz)
            # before the real ones-chains start.
            for w in range(24):
                nc.tensor.matmul(warm[0:1, 0:1], ones_f[:], gam_t[:],
                                 start=True, stop=True)

            first = True
            for ci, (s, c0, cols, ka) in enumerate(chunks):
                r0, r1 = s * P, (s + 1) * P
                c1 = c0 + cols
                t_t = tp.tile([P, FALLOC], bf16, name=f"t_{ci}", tag="t")
                x0_t = x0p.tile([P, FALLOC], bf16, name=f"x0_{ci}", tag="x0")
                x1_t = x1p.tile([P, FALLOC], bf16, name=f"x1_{ci}", tag="x1")
                nc.sync.dma_start(t_t[:, 0:cols], tg[r0:r1, c0:c1])
                nc.sync.dma_start(x0_t[:, 0:cols], x0[r0:r1, c0:c1])
                nc.sync.dma_start(x1_t[:, 0:cols], x1[r0:r1, c0:c1])

                Vc = vcp.tile([P, 16], f32, name=f"vc_{ci}", tag="vc")
                nc.gpsimd.memset(Vc[:], 0.0)

                if first:
                    # out2 warmup: tiny abs+sigmoid forces the sigmoid
                    # table load before the first bulk ACT op.
                    nc.sync.dma_start(o2_t[:], o2[0:1, :])
                    nc.vector.scalar_tensor_tensor(
                        out=a_o2[:], in0=o2_t[:], scalar=-1.0, in1=o2_t[:],
                        op0=ALU.mult, op1=ALU.max,
                    )
                    nc.vector.tensor_scalar(
                        out=r_o2[:], in0=o2_t[:], scalar1=0.0, scalar2=None,
                        op0=ALU.max, op1=ALU.add,
                        accum_out=Vc[0:1, 4:5],
                    )
                    nc.scalar.activation(
                        a_o2[:], a_o2[:], AFT.Sigmoid,
                        bias=gam_t[0:1], scale=-BETA,
                        accum_out=Vc[0:1, 5:6],
                    )
                    first = False

                s_t = sp.tile([P, FALLOC], bf16, name=f"s_{ci}", tag="s")
                nc.vector.tensor_scalar(
                    out=s_t[:, 0:cols], in0=t_t[:, 0:cols], scalar1=0.5,
                    scalar2=None, op0=ALU.subtract,
                )
                xts = (x0_t, x1_t)
                rn_ts = [rnp.tile([P, FALLOC], bf16, name=f"rn{i}_{ci}", tag="rn")
                         for i in range(2)]
                # phase 1 (DVE): z'' = s_t * x, in place over x
                for i, xt in enumerate(xts):
                    nc.vector.tensor_tensor(
                        xt[:, 0:cols], s_t[:, 0:cols], xt[:, 0:cols], ALU.mult
                    )
                # phase 2 (ACT): rn = relu(-2 z'') on front ka cols with an
                # exact fused sum; (DVE): same on back cols via 2-op ts.
                for i, xt in enumerate(xts):
                    if ka:
                        nc.scalar.activation(
                            rn_ts[i][:, 0:ka], xt[:, 0:ka], AFT.Relu,
                            scale=-2.0, accum_out=Vc[:, 2 + i : 3 + i],
                        )
                for i, xt in enumerate(xts):
                    nc.vector.tensor_scalar(
                        out=rn_ts[i][:, ka:cols], in0=xt[:, ka:cols],
                        scalar1=-2.0, scalar2=0.0, op0=ALU.mult, op1=ALU.max,
                    )
                # phase 3: a = z'' + rn = |z''| in place; then sigma in place
                for i, xt in enumerate(xts):
                    nc.vector.tensor_tensor(
                        xt[:, 0:cols], xt[:, 0:cols], rn_ts[i][:, 0:cols],
                        ALU.add
                    )
                    nc.scalar.activation(
                        xt[:, 0:cols], xt[:, 0:cols], AFT.Sigmoid,
                        bias=gam_t[:], scale=-2.0 * BETA,
                        accum_out=Vc[:, i : i + 1],
                    )
                # phase 4 (PE): chains over the back rn cols
                for i in range(2):
                    for j in range(ka, cols, TBLK):
                        nc.tensor.matmul(
                            pr[i][:], ones_b[:], rn_ts[i][:, j : j + TBLK],
                            start=(rcols[i] == 0),
                            stop=(rcols[i] + TBLK == rtot),
                        )
                        rcols[i] += TBLK

                # PE chain: per-sample sum(t)
                for j in range(cols // TBLK):
                    off = c0 + j * TBLK
                    nc.tensor.matmul(
                        pt[s][:], ones_b[:], t_t[:, j * TBLK : (j + 1) * TBLK],
                        start=(off == 0), stop=(off + TBLK == FREE_PER_SAMPLE),
                    )

                # fold this chunk's stats into PSUM (fp32r ones-matmul)
                nc.tensor.matmul(
                    U[:], ones_f[:], Vc[:],
                    start=(ci == 0), stop=(ci == n_chunks - 1),
                )

                if ci == N_CHUNKS0 - 1:
                    # sample 0's t-chain just stopped; drain it early
                    nc.vector.tensor_reduce(
                        out=Us[0:1, 8:9], in_=pt[0][:],
                        axis=mybir.AxisListType.X, op=ALU.add,
                    )

            nc.vector.tensor_reduce(
                out=Us[0:1, 9:10], in_=pt[1][:],
                axis=mybir.AxisListType.X, op=ALU.add,
            )
            for i in range(2):
                nc.vector.tensor_reduce(
                    out=Us[0:1, 10 + i : 11 + i], in_=pr[i][:],
                    axis=mybir.AxisListType.X, op=ALU.add,
                )
            nc.vector.tensor_copy(Us[0:1, 0:8], U[0:1, 0:8])
            nc.sync.dma_start(res[0:1, :], Us[:])

    nc.finalize()
    return nc


def _get_nc():
    if "nc" not in _CACHE:
        _CACHE["nc"] = _build()
    return _CACHE["nc"]


def _run(in_maps, trace=False):
    from concourse.bass_utils import run_bass_kernel_spmd

    return run_bass_kernel_spmd(
        _get_nc(), in_maps, core_ids=list(range(N_CORES)), trace=trace
    )


def make_in_maps(out0, out1, out2, targets):
    bf = ml_dtypes.bfloat16
    out0 = np.asarray(out0, dtype=np.float32).astype(bf)
    out1 = np.asarray(out1, dtype=np.float32).astype(bf)
    targets = np.asarray(targets, dtype=np.float32).astype(bf)
    out2 = np.asarray(out2, dtype=np.float32)
    in_maps = []
    for c in range(N_CORES):
        sl = slice(c * B_LOCAL, (c + 1) * B_LOCAL)
        in_maps.append(
            {
                "out0": np.ascontiguousarray(out0[sl]).reshape(ROWS, FREE_PER_SAMPLE),
                "out1": np.ascontiguousarray(out1[sl]).reshape(ROWS, FREE_PER_SAMPLE),
                "targets": np.ascontiguousarray(targets[sl]).reshape(
                    ROWS, FREE_PER_SAMPLE
                ),
                "out2": np.ascontiguousarray(out2[sl]).reshape(1, B_LOCAL * C),
            }
        )
    return in_maps


def combine_partials(stats, out2):
    """Host-side O(1) combine. stats: [N_CORES, 16] per-core sums.
    cols: 0,1=sum sig per tensor; 2,3=ACT-side sum relu(z) per tensor;
    4=sum relu(o2); 5=sum sig(o2); 8,9=sum(t) per sample;
    10,11=PE-side sum relu(z) per tensor."""
    total_main = 0.0
    total_se = 0.0
    for c in range(len(stats)):
        v = [float(x) for x in stats[c]]
        s0 = v[2] + v[10] + ALPHA * v[0] + DELTA * N_CORE_T
        s1 = v[3] + v[11] + ALPHA * v[1] + DELTA * N_CORE_T
        total_main += s0 + AUX_WEIGHT * s1
        total_se += v[4] + ALPHA * v[5] + DELTA * (B_LOCAL * C)
        for s in range(B_LOCAL):
            t_sum = v[8 + s]
            b_global = c * B_LOCAL + s
            if t_sum < ELEMS_PER_SAMPLE - 0.5:  # class-bin 0 present
                total_se -= float(out2[b_global, 0])
            if t_sum > 0.5:  # class-bin 1 present
                total_se -= float(out2[b_global, 1])
    return total_main / N_TOTAL + SE_WEIGHT * total_se / N_SE


def kernel(out0, out1, out2, targets):
    out2 = np.asarray(out2, dtype=np.float32)
    br = _run(make_in_maps(out0, out1, out2, targets))
    stats = [r["stats"][0] for r in br.results]
    return np.asarray(combine_partials(stats, out2), dtype=np.float32)


# revision 18
# speedup vs baseline: 1.0238x; 1.0140x over previous
"""AuxSeLoss v7c: bf16 upload, engine-balanced sigmoid+relu softplus.

Math: t in {0,1} exactly -> per-element BCE = softplus(z), z = (1-2t)x.
softplus(z) = relu(z) + g(|z|), g(u) = log1p(exp(-u)) ~= A*sigmoid(-B*u+G)+D
(minimax fit, max |err| 4.9e-4).  With z'' = (t-0.5)x (so z = -2z''):
relu(z) = rn := max(-2z'', 0) and |z''| = z'' + rn.

HW facts this build is shaped by (all measured on-device):
- DVE modes: plain tensor_tensor 2x, plain tensor_scalar (incl 2-op
  form) 4x, but EVERY fused-accum form (stt/ts+accum/custom) is 1x, and
  abs_max/bitwise ops are ISA-rejected -> DVE does tiles only, no sums.
- ACT is 1 elem/cycle with a free exact fp32 accumulator (~0.7us/op
  overhead); PE ones-chains cost ~0.4-0.6us per 512-col block (max
  N=512 even for bf16; HAM clock starts cold at 1.2GHz).
Dataflow per chunk (bf16 tiles, in place over the x tile):
  DVE: s_t = t-0.5 [ts];  z''= s_t*x [tt];  rn back-cols [2-op ts];
       a = z''+rn = |z''| [tt];   ACT: relu(-2 z'') on front ka cols
  (accum = that slice's exact sum relu(z)) + one Sigmoid pass
  (scale=-2B) per tensor (accum = sum sigma);  PE: 512-col ones-chains
  for per-sample sum(t) + back-cols sum(rn), per-chunk Vc fold, and a
  burst of tiny warm-up matmuls at t=0 to heat the HAM clock gate.
  DMA: 16.5 MB/core bf16 on the sync HWDGE queue (~44.5us).
Result: 121us (v4 exp/ln baseline) -> ~87-89us, rel err 9.7e-5.
DVE is the saturated engine (~66us busy, 100% from 20-80us); ~7us is
fixed framework preamble before the first DMA can even trigger.
Host: f32->bf16 cast (ml_dtypes, round-to-nearest) + O(1) combine.
"""

import numpy as np
import ml_dtypes

N_CLASSES = 21
B, C, H, W = 16, N_CLASSES, 256, 256
N_CORES = 8
B_LOCAL = B // N_CORES
ELEMS_PER_SAMPLE = C * H * W  # 1376256
P = 128
FREE_PER_SAMPLE = ELEMS_PER_SAMPLE // P  # 10752
ROWS = B_LOCAL * P
AUX_WEIGHT = 0.4
SE_WEIGHT = 0.2
N_TOTAL = B * C * H * W
N_SE = B * C
N_CORE_T = B_LOCAL * ELEMS_PER_SAMPLE  # elems per tensor per core

# minimax fit of log1p(exp(-u)) ~= ALPHA*sigmoid(-BETA*u+GAMMA)+DELTA, u>=0
ALPHA = 2.49059269
BETA = 0.985901754
GAMMA = -0.954224925
DELTA = 1.79845165e-4

# (cols, act_relu_cols) per chunk; act cols ride ACT, rest DVE+PE.
CHUNK_SCHEDULE = [
    [(1024, 0), (2048, 512), (3072, 1024), (4608, 1536)],
    [(5632, 1536), (4608, 1024), (512, 0)],
]
assert all(sum(c for c, _ in cs) == FREE_PER_SAMPLE for cs in CHUNK_SCHEDULE)
assert all(c % 512 == 0 and k % 512 == 0 and k < c
           for cs in CHUNK_SCHEDULE for c, k in cs)
N_CHUNKS0 = len(CHUNK_SCHEDULE[0])
FALLOC = 5776  # 5632 max cols + 288B pad so bank phases differ
TBLK = 512  # PE chain block width (one PSUM bank of f32)

_CACHE: dict = {}


def _build():
    from contextlib import ExitStack

    import concourse.bacc as bacc
    import concourse.mybir as mybir
    from concourse.tile import TileContext

    f32 = mybir.dt.float32
    bf16 = mybir.dt.bfloat16
    AFT = mybir.ActivationFunctionType
    ALU = mybir.AluOpType

    nc = bacc.Bacc("TRN2", target_bir_lowering=False)
    x0 = nc.dram_tensor("out0", [ROWS, FREE_PER_SAMPLE], bf16, kind="ExternalInput")
    x1 = nc.dram_tensor("out1", [ROWS, FREE_PER_SAMPLE], bf16, kind="ExternalInput")
    tg = nc.dram_tensor("targets", [ROWS, FREE_PER_SAMPLE], bf16, kind="ExternalInput")
    o2 = nc.dram_tensor("out2", [1, B_LOCAL * C], f32, kind="ExternalInput")
    res = nc.dram_tensor("stats", [1, 16], f32, kind="ExternalOutput")

    with ExitStack() as ctx, TileContext(nc) as tc:
        with (
            tc.tile_pool(name="x0p", bufs=3) as x0p,
            tc.tile_pool(name="x1p", bufs=3) as x1p,
            tc.tile_pool(name="tp", bufs=3) as tp,
            tc.tile_pool(name="sp", bufs=2) as sp,
            tc.tile_pool(name="rnp", bufs=4) as rnp,
            tc.tile_pool(name="vcp", bufs=2) as vcp,
            tc.tile_pool(name="accp", bufs=1) as accp,
            tc.tile_pool(name="psp", bufs=1, space="PSUM") as psp,
        ):
            ones_f = accp.tile([P, 1], f32)
            nc.vector.memset(ones_f[:], 1.0)
            ones_b = accp.tile([P, 1], bf16)
            nc.vector.memset(ones_b[:], 1.0)
            gam_t = accp.tile([P, 1], f32)
            nc.vector.memset(gam_t[:], GAMMA)
            Us = accp.tile([1, 16], f32)
            o2_t = accp.tile([1, B_LOCAL * C], f32)
            a_o2 = accp.tile([1, B_LOCAL * C], f32)
            r_o2 = accp.tile([1, B_LOCAL * C], f32)
            U = psp.tile([1, 16], f32)
            warm = psp.tile([1, 16], f32)
            pt = [psp.tile([1, TBLK], f32, name=f"pt{s}") for s in range(B_LOCAL)]
            pr = [psp.tile([1, TBLK], f32, name=f"pr{i}") for i in range(2)]

            chunks = []
            for s in range(B_LOCAL):
                c0 = 0
                for cols, ka in CHUNK_SCHEDULE[s]:
                    chunks.append((s, c0, cols, ka))
                    c0 += cols
            n_chunks = len(chunks)
            # total PE rn-chain cols per tensor (back cols of every chunk)
            rtot = sum(c - k for cs in CHUNK_SCHEDULE for c, k in cs)
            rcols = [0, 0]

            # HAM warm-up: ~4.5us of back-to-back tiny matmuls while the
            # DMA pipe fills; flips the PE clock gate to 8/8 (2.4 GHz)
            # before the real ones-chains start.
            for w in range(24):
                nc.tensor.matmul(warm[0:1, 0:1], ones_f[:], gam_t[:],
                                 start=True, stop=True)

            first = True
            for ci, (s, c0, cols, ka) in enumerate(chunks):
                r0, r1 = s * P, (s + 1) * P
                c1 = c0 + cols
                t_t = tp.tile([P, FALLOC], bf16, name=f"t_{ci}", tag="t")
                x0_t = x0p.tile([P, FALLOC], bf16, name=f"x0_{ci}", tag="x0")
                x1_t = x1p.tile([P, FALLOC], bf16, name=f"x1_{ci}", tag="x1")
                nc.sync.dma_start(t_t[:, 0:cols], tg[r0:r1, c0:c1])
                nc.sync.dma_start(x0_t[:, 0:cols], x0[r0:r1, c0:c1])
                nc.sync.dma_start(x1_t[:, 0:cols], x1[r0:r1, c0:c1])

                Vc = vcp.tile([P, 16], f32, name=f"vc_{ci}", tag="vc")
                nc.gpsimd.memset(Vc[:], 0.0)

                if first:
                    # out2 warmup: tiny abs+sigmoid forces the sigmoid
                    # table load before the first bulk ACT op.
                    nc.sync.dma_start(o2_t[:], o2[0:1, :])
                    nc.vector.scalar_tensor_tensor(
                        out=a_o2[:], in0=o2_t[:], scalar=-1.0, in1=o2_t[:],
                        op0=ALU.mult, op1=ALU.max,
                    )
                    nc.vector.tensor_scalar(
                        out=r_o2[:], in0=o2_t[:], scalar1=0.0, scalar2=None,
                        op0=ALU.max, op1=ALU.add,
                        accum_out=Vc[0:1, 4:5],
                    )
                    nc.scalar.activation(
                        a_o2[:], a_o2[:], AFT.Sigmoid,
                        bias=gam_t[0:1], scale=-BETA,
                        accum_out=Vc[0:1, 5:6],
                    )
                    first = False

                s_t = sp.tile([P, FALLOC], bf16, name=f"s_{ci}", tag="s")
                nc.vector.tensor_scalar(
                    out=s_t[:, 0:cols], in0=t_t[:, 0:cols], scalar1=0.5,
                    scalar2=None, op0=ALU.subtract,
                )
                xts = (x0_t, x1_t)
                rn_ts = [rnp.tile([P, FALLOC], bf16, name=f"rn{i}_{ci}", tag="rn")
                         for i in range(2)]
                # phase 1 (DVE): z'' = s_t * x, in place over x
                for i, xt in enumerate(xts):
                    nc.vector.tensor_tensor(
                        xt[:, 0:cols], s_t[:, 0:cols], xt[:, 0:cols], ALU.mult
                    )
                # phase 2 (ACT): rn = relu(-2 z'') on front ka cols with an
                # exact fused sum; (DVE): same on back cols via 2-op ts.
                for i, xt in enumerate(xts):
                    if ka:
                        nc.scalar.activation(
                            rn_ts[i][:, 0:ka], xt[:, 0:ka], AFT.Relu,
                            scale=-2.0, accum_out=Vc[:, 2 + i : 3 + i],
                        )
                for i, xt in enumerate(xts):
                    nc.vector.tensor_scalar(
                        out=rn_ts[i][:, ka:cols], in0=xt[:, ka:cols],
                        scalar1=-2.0, scalar2=0.0, op0=ALU.mult, op1=ALU.max,
                    )
                # phase 3: a = z'' + rn = |z''| in place; then sigma in place
                for i, xt in enumerate(xts):
                    nc.vector.tensor_tensor(
                        xt[:, 0:cols], xt[:, 0:cols], rn_ts[i][:, 0:cols],
                        ALU.add
                    )
                    nc.scalar.activation(
                        xt[:, 0:cols], xt[:, 0:cols], AFT.Sigmoid,
                        bias=gam_t[:], scale=-2.0 * BETA,
                        accum_out=Vc[:, i : i + 1],
                    )
                # phase 4 (PE): chains over the back rn cols
                for i in range(2):
                    for j in range(ka, cols, TBLK):
                        nc.tensor.matmul(
                            pr[i][:], ones_b[:], rn_ts[i][:, j : j + TBLK],
                            start=(rcols[i] == 0),
                            stop=(rcols[i] + TBLK == rtot),
                        )
                        rcols[i] += TBLK

                # PE chain: per-sample sum(t)
                for j in range(cols // TBLK):
                    off = c0 + j * TBLK
                    nc.tensor.matmul(
                        pt[s][:], ones_b[:], t_t[:, j * TBLK : (j + 1) * TBLK],
                        start=(off == 0), stop=(off + TBLK == FREE_PER_SAMPLE),
                    )

                # fold this chunk's stats into PSUM (fp32r ones-matmul)
                nc.tensor.matmul(
                    U[:], ones_f[:], Vc[:],
                    start=(ci == 0), stop=(ci == n_chunks - 1),
                )

                if ci == N_CHUNKS0 - 1:
                    # sample 0's t-chain just stopped; drain it early
                    nc.vector.tensor_reduce(
                        out=Us[0:1, 8:9], in_=pt[0][:],
                        axis=mybir.AxisListType.X, op=ALU.add,
                    )

            nc.vector.tensor_reduce(
                out=Us[0:1, 9:10], in_=pt[1][:],
                axis=mybir.AxisListType.X, op=ALU.add,
            )
            for i in range(2):
                nc.vector.tensor_reduce(
                    out=Us[0:1, 10 + i : 11 + i], in_=pr[i][:],
                    axis=mybir.AxisListType.X, op=ALU.add,
                )
            nc.vector.tensor_copy(Us[0:1, 0:8], U[0:1, 0:8])
            nc.sync.dma_start(res[0:1, :], Us[:])

    nc.finalize()
    return nc


def _get_nc():
    if "nc" not in _CACHE:
        _CACHE["nc"] = _build()
    return _CACHE["nc"]


def _run(in_maps, trace=False):
    from concourse.bass_utils import run_bass_kernel_spmd

    return run_bass_kernel_spmd(
        _get_nc(), in_maps, core_ids=list(range(N_CORES)), trace=trace
    )


def make_in_maps(out0, out1, out2, targets):
    bf = ml_dtypes.bfloat16
    out0 = np.asarray(out0, dtype=np.float32).astype(bf)
    out1 = np.asarray(out1, dtype=np.float32).astype(bf)
    targets = np.asarray(targets, dtype=np.float32).astype(bf)
    out2 = np.asarray(out2, dtype=np.float32)
    in_maps = []
    for c in range(N_CORES):
        sl = slice(c * B_LOCAL, (c + 1) * B_LOCAL)
        in_maps.append(
            {
                "out0": np.ascontiguousarray(out0[sl]).reshape(ROWS, FREE_PER_SAMPLE),
                "out1": np.ascontiguousarray(out1[sl]).reshape(ROWS, FREE_PER_SAMPLE),
                "targets": np.ascontiguousarray(targets[sl]).reshape(
                    ROWS, FREE_PER_SAMPLE
                ),
                "out2": np.ascontiguousarray(out2[sl]).reshape(1, B_LOCAL * C),
            }
        )
    return in_maps


def combine_partials(stats, out2):
    """Host-side O(1) combine. stats: [N_CORES, 16] per-core sums.
    cols: 0,1=sum sig per tensor; 2,3=ACT-side sum relu(z) per tensor;
    4=sum relu(o2); 5=sum sig(o2); 8,9=sum(t) per sample;
    10,11=PE-side sum relu(z) per tensor."""
    total_main = 0.0
    total_se = 0.0
    for c in range(len(stats)):
        v = [float(x) for x in stats[c]]
        s0 = v[2] + v[10] + ALPHA * v[0] + DELTA * N_CORE_T
        s1 = v[3] + v[11] + ALPHA * v[1] + DELTA * N_CORE_T
        total_main += s0 + AUX_WEIGHT * s1
        total_se += v[4] + ALPHA * v[5] + DELTA * (B_LOCAL * C)
        for s in range(B_LOCAL):
            t_sum = v[8 + s]
            b_global = c * B_LOCAL + s
            if t_sum < ELEMS_PER_SAMPLE - 0.5:  # class-bin 0 present
                total_se -= float(out2[b_global, 0])
            if t_sum > 0.5:  # class-bin 1 present
                total_se -= float(out2[b_global, 1])
    return total_main / N_TOTAL + SE_WEIGHT * total_se / N_SE


def kernel(out0, out1, out2, targets):
    out2 = np.asarray(out2, dtype=np.float32)
    br = _run(make_in_maps(out0, out1, out2, targets))
    stats = [r["stats"][0] for r in br.results]
    return np.asarray(combine_partials(stats, out2), dtype=np.float32)
